# revision 16
# baseline (speedup 1.0000x reference)
"""Diagonal-covariance MVN negative log-likelihood loss on 8 TRN2 NeuronCores.

loss = -(1/B) * sum_b log_prob_b
     = 0.5 * ( sum_{b,d} [ (t-mu)^2/sigma + ln(sigma) ] / B  +  D*ln(2pi) )

Sharding: pure data parallel over the batch dim (B=16384 -> 2048 rows/core).
Each core streams its 3x16MB shard through a raw-Bass 3-engine pipeline
(SP issues DMA loads, ACT does ln/exp/square with free row-sum accumulation,
DVE does subtract/multiply), and outputs a tiny (128, 32) stats tile of
per-partition partial sums. The final scalar reduction happens on the host
in float64.

Raw Bass (not Tile) because this toolchain's walrus rejects instructions
carrying more than one attached sync wait; manual standalone wait_ge
instructions sidestep that. (It also rejects DMA accum_op=subtract and
tensor_tensor_reduce at codegen, which constrains the usable op set to
activation (+accum_out) / tensor_sub / tensor_mul + plain DMA.)

Active design: build_nc_v2 (BUILDER below). Optimization survey on this
hardware (interleaved paired differential benches at R=199 repeats,
SE ~ +/-2.4us/round) measured every structural alternative equal or
worse: balanced HWDGE rings (v4), 3-stream with Pool SWDGE (v5),
DVE tensor_tensor_reduce pipelines (v6/v7, rejected by walrus anyway),
DMA-folded subtract (v9, rejected by walrus), ACT-DMA-free restructure
with split trailing tiles (v10: +3..+10us/round depending on config),
deeper buffering (nb=4: +7us/round). v2's unbarriered steady-state is
135.6us/pass for 48MB/core = 354 GB/s sustained aggregate DMA (~99% of
the ~360 GB/s per-core cap), so the kernel is at the memory roofline;
the ~11us single-shot overhead over steady-state resists structural
attack (it is NEFF startup/ramp mechanics, not compute tail).
"""

import sys
from contextlib import ExitStack

for _p in ("/opt/trn_rl_repo", "/opt/pypackages"):
    if _p not in sys.path:
        sys.path.insert(0, _p)

import numpy as np

import concourse.bass as bass
import concourse.mybir as mybir
from concourse.bass_utils import run_bass_kernel_spmd

B, D = 16384, 2048
N_CORES = 8
RPC = B // N_CORES          # rows per core = 2048
P = 128                     # SBUF partitions
NT = RPC // P               # 16 row-tiles per core
NB = 2                      # buffers per stream (double buffering)
LOG_2PI = float(np.log(2.0 * np.pi))

TRACE = False
LAST_RESULTS = None

_nc_cache = None


def build_nc(repeats: int = 1) -> bass.Bass:
    """repeats>1 re-runs the identical body R times (idempotent: activation
    accum_out overwrites) — used only by the benchmark's differential timing."""
    nc = bass.Bass()
    f32 = mybir.dt.float32
    F = mybir.ActivationFunctionType
    mu = nc.dram_tensor("mu", [RPC, D], f32, kind="ExternalInput")
    sg = nc.dram_tensor("sigma", [RPC, D], f32, kind="ExternalInput")
    tg = nc.dram_tensor("target", [RPC, D], f32, kind="ExternalInput")
    # stats[:, 0:NT]   = per-partition sums of ln(sigma) for tile i
    # stats[:, NT:2NT] = per-partition sums of (t-mu)^2/sigma for tile i
    stats = nc.dram_tensor("stats", [P, 2 * NT], f32, kind="ExternalOutput")

    mu3 = mu[:, :].rearrange("(n p) d -> n p d", p=P)
    sg3 = sg[:, :].rearrange("(n p) d -> n p d", p=P)
    tg3 = tg[:, :].rearrange("(n p) d -> n p d", p=P)

    with ExitStack() as ctx:
        def bufs(name):
            return [
                ctx.enter_context(nc.sbuf_tensor(f"{name}{j}", [P, D], f32))
                for j in range(NB)
            ]

        sgt, mut, tgt = bufs("sgt"), bufs("mut"), bufs("tgt")
        lt, rst, dft, wt, qt = bufs("lt"), bufs("rst"), bufs("dft"), bufs("wt"), bufs("qt")
        stats_t = ctx.enter_context(nc.sbuf_tensor("stats_t", [P, 2 * NT], f32))

        # One sem per (stream, buffer slot): at most one in-flight increment
        # each, so waits always target the sem's final value (HWDGE
        # completions across dma_starts are not ordered).
        sg_sem = [ctx.enter_context(nc.semaphore(f"sg_sem{j}")) for j in range(NB)]
        mu_sem = [ctx.enter_context(nc.semaphore(f"mu_sem{j}")) for j in range(NB)]
        tg_sem = [ctx.enter_context(nc.semaphore(f"tg_sem{j}")) for j in range(NB)]
        asem = ctx.enter_context(nc.semaphore("asem"))   # +1 per ACT op (3/iter)
        vsem = ctx.enter_context(nc.semaphore("vsem"))   # +1 per DVE op (2/iter)
        ssem = ctx.enter_context(nc.semaphore("ssem"))   # +16 final store
        block = ctx.enter_context(nc.Block())

        NK = repeats * NT

        @block.sync
        def _(sync):
            for k in range(NK):
                i, p = k % NT, k % NB
                if k >= NB:
                    # buffer recycle: iter k-NB consumers must be done
                    sync.wait_ge(asem, 3 * (k - NB) + 1)   # Ln_{k-NB} read sgt[p]
                    sync.wait_ge(vsem, 2 * (k - NB) + 1)   # sub_{k-NB} read mut/tgt[p]
                sync.dma_start(out=sgt[p][:, :], in_=sg3[i, :, :]).then_inc(sg_sem[p], 16)
                sync.dma_start(out=mut[p][:, :], in_=mu3[i, :, :]).then_inc(mu_sem[p], 16)
                sync.dma_start(out=tgt[p][:, :], in_=tg3[i, :, :]).then_inc(tg_sem[p], 16)
            sync.wait_ge(asem, 3 * NK)                     # all ACT done
            sync.dma_start(out=stats[:, :], in_=stats_t[:, :]).then_inc(ssem, 16)
            sync.wait_ge(ssem, 16)

        @block.scalar
        def _(scalar):
            for k in range(NK):
                i, p = k % NT, k % NB
                scalar.wait_ge(sg_sem[p], 16 * (k // NB + 1))  # sigma_k loaded
                nc.scalar.activation(
                    lt[p][:, :], sgt[p][:, :], F.Ln,
                    accum_out=stats_t[:, i : i + 1],
                ).then_inc(asem, 1)                        # tick 3k+1
                if k >= NB:
                    scalar.wait_ge(vsem, 2 * (k - NB) + 2)  # mul_{k-NB} read rst[p]
                scalar.wait_ge(asem, 3 * k + 1)            # Ln_k wrote lt[p] (same-engine RAW)
                nc.scalar.activation(
                    rst[p][:, :], lt[p][:, :], F.Exp, scale=-0.5,
                ).then_inc(asem, 1)                        # tick 3k+2
                scalar.wait_ge(vsem, 2 * k + 2)            # mul_k wrote wt[p]
                nc.scalar.activation(
                    qt[p][:, :], wt[p][:, :], F.Square,
                    accum_out=stats_t[:, NT + i : NT + i + 1],
                ).then_inc(asem, 1)                        # tick 3k+3

        @block.vector
        def _(vector):
            for k in range(NK):
                p = k % NB
                vector.wait_ge(mu_sem[p], 16 * (k // NB + 1))  # mu_k loaded
                vector.wait_ge(tg_sem[p], 16 * (k // NB + 1))  # tg_k loaded
                nc.vector.tensor_sub(
                    dft[p][:, :], tgt[p][:, :], mut[p][:, :]
                ).then_inc(vsem, 1)                        # tick 2k+1
                vector.wait_ge(asem, 3 * k + 2)            # Exp_k wrote rst[p]
                vector.wait_ge(vsem, 2 * k + 1)            # sub_k wrote dft[p] (same-engine RAW)
                nc.vector.tensor_mul(
                    wt[p][:, :], dft[p][:, :], rst[p][:, :]
                ).then_inc(vsem, 1)                        # tick 2k+2

    return nc


def build_nc_v2(repeats: int = 1, nb: int = 3) -> bass.Bass:
    """v2: 4 SBUF tile groups with buffer reuse, nb-deep pipelining, sigma
    loads issued from the ACT engine's own HWDGE ring (mu/target on SP's),
    in-place DVE ops.

    Per iteration k (slot p = k % nb), tile index i = k % NT:
      SP :  load mut[p] <- mu_i, tgt[p] <- tg_i        (after Square_{k-nb})
      ACT:  Ln:  lt[p] <- ln(sgt[p])          accum -> stats[:, i]
            Exp: sgt[p] <- exp(-0.5*lt[p])    (rs overwrites sigma)
            Square: mut[p] <- (tgt[p])^2      accum -> stats[:, NT+i]
            issue load sgt[p] <- sg_{k+nb}    (rs dead after mul_k)
      DVE:  sub: tgt[p] <- tgt[p] - mut[p]    (diff, in place)
            mul: tgt[p] <- tgt[p] * sgt[p]    (w = diff * rs, in place)
    """
    nc = bass.Bass()
    f32 = mybir.dt.float32
    F = mybir.ActivationFunctionType
    mu = nc.dram_tensor("mu", [RPC, D], f32, kind="ExternalInput")
    sg = nc.dram_tensor("sigma", [RPC, D], f32, kind="ExternalInput")
    tg = nc.dram_tensor("target", [RPC, D], f32, kind="ExternalInput")
    stats = nc.dram_tensor("stats", [P, 2 * NT], f32, kind="ExternalOutput")

    mu3 = mu[:, :].rearrange("(n p) d -> n p d", p=P)
    sg3 = sg[:, :].rearrange("(n p) d -> n p d", p=P)
    tg3 = tg[:, :].rearrange("(n p) d -> n p d", p=P)

    NK = repeats * NT

    with ExitStack() as ctx:
        def bufs(name):
            return [
                ctx.enter_context(nc.sbuf_tensor(f"{name}{j}", [P, D], f32))
                for j in range(nb)
            ]

        sgt, mut, tgt, lt = bufs("sgt"), bufs("mut"), bufs("tgt"), bufs("lt")
        stats_t = ctx.enter_context(nc.sbuf_tensor("stats_t", [P, 2 * NT], f32))
        warm_t = ctx.enter_context(nc.sbuf_tensor("warm_t", [P, 1], f32))

        sg_sem = [ctx.enter_context(nc.semaphore(f"sg_sem{j}")) for j in range(nb)]
        mu_sem = [ctx.enter_context(nc.semaphore(f"mu_sem{j}")) for j in range(nb)]
        tg_sem = [ctx.enter_context(nc.semaphore(f"tg_sem{j}")) for j in range(nb)]
        asem = ctx.enter_context(nc.semaphore("asem"))   # +1 per activation
        vsem = ctx.enter_context(nc.semaphore("vsem"))   # +1 per DVE op
        ssem = ctx.enter_context(nc.semaphore("ssem"))   # +16 final store
        block = ctx.enter_context(nc.Block())

        one_ap = nc.const_aps.tensor(1.0, (P, 1), f32)

        @block.sync
        def _(sync):
            for k in range(NK):
                i, p = k % NT, k % nb
                if k >= nb:
                    # Square_{k-nb} read tgt[p] (w) and wrote mut[p] (q):
                    # both buffers free once it completes.
                    sync.wait_ge(asem, 3 * (k - nb) + 4)
                sync.dma_start(out=mut[p][:, :], in_=mu3[i, :, :]).then_inc(mu_sem[p], 16)
                sync.dma_start(out=tgt[p][:, :], in_=tg3[i, :, :]).then_inc(tg_sem[p], 16)
            sync.wait_ge(asem, 3 * NK + 1)
            sync.dma_start(out=stats[:, :], in_=stats_t[:, :]).then_inc(ssem, 16)
            sync.wait_ge(ssem, 16)

        @block.scalar
        def _(scalar):
            # table prewarm: no waits, so the ~2.7us ACT table load
            # overlaps the DMA fill instead of serializing after sigma_0.
            nc.scalar.activation(warm_t[:, :], one_ap, F.Ln).then_inc(asem, 1)  # tick 1
            # prologue: first nb sigma loads on the ACT ring
            for j in range(min(nb, NK)):
                nc.scalar.dma_start(
                    out=sgt[j][:, :], in_=sg3[j % NT, :, :]
                ).then_inc(sg_sem[j], 16)
            for k in range(NK):
                i, p = k % NT, k % nb
                scalar.wait_ge(sg_sem[p], 16 * (k // nb + 1))   # sigma_k landed
                nc.scalar.activation(
                    lt[p][:, :], sgt[p][:, :], F.Ln,
                    accum_out=stats_t[:, i : i + 1],
                ).then_inc(asem, 1)                             # tick 3k+2
                scalar.wait_ge(asem, 3 * k + 2)                 # Ln_k done (RAW lt)
                nc.scalar.activation(
                    sgt[p][:, :], lt[p][:, :], F.Exp, scale=-0.5,
                ).then_inc(asem, 1)                             # tick 3k+3
                scalar.wait_ge(vsem, 2 * k + 2)                 # mul_k done (w ready, rs dead)
                nc.scalar.activation(
                    mut[p][:, :], tgt[p][:, :], F.Square,
                    accum_out=stats_t[:, NT + i : NT + i + 1],
                ).then_inc(asem, 1)                             # tick 3k+4
                if k + nb < NK:
                    nc.scalar.dma_start(
                        out=sgt[p][:, :], in_=sg3[(k + nb) % NT, :, :]
                    ).then_inc(sg_sem[p], 16)

        @block.vector
        def _(vector):
            for k in range(NK):
                p = k % nb
                vector.wait_ge(mu_sem[p], 16 * (k // nb + 1))
                vector.wait_ge(tg_sem[p], 16 * (k // nb + 1))
                nc.vector.tensor_sub(
                    tgt[p][:, :], tgt[p][:, :], mut[p][:, :]
                ).then_inc(vsem, 1)                             # tick 2k+1
                vector.wait_ge(asem, 3 * k + 3)                 # Exp_k done (rs ready)
                vector.wait_ge(vsem, 2 * k + 1)                 # sub_k done (RAW)
                nc.vector.tensor_mul(
                    tgt[p][:, :], tgt[p][:, :], sgt[p][:, :]
                ).then_inc(vsem, 1)                             # tick 2k+2

    return nc


def build_nc_v4(repeats: int = 1, nb: int = 3) -> bass.Bass:
    """v4: v2 with balanced DMA rings — target loads alternate between the
    SP ring (even k) and the ACT ring (odd k), evening traffic to 24MB/24MB
    per pass instead of 32/16. ACT-issued target loads piggyback on the same
    gating as the sigma reloads (Square_k completion frees tgt[p])."""
    nc = bass.Bass()
    f32 = mybir.dt.float32
    F = mybir.ActivationFunctionType
    mu = nc.dram_tensor("mu", [RPC, D], f32, kind="ExternalInput")
    sg = nc.dram_tensor("sigma", [RPC, D], f32, kind="ExternalInput")
    tg = nc.dram_tensor("target", [RPC, D], f32, kind="ExternalInput")
    stats = nc.dram_tensor("stats", [P, 2 * NT], f32, kind="ExternalOutput")

    mu3 = mu[:, :].rearrange("(n p) d -> n p d", p=P)
    sg3 = sg[:, :].rearrange("(n p) d -> n p d", p=P)
    tg3 = tg[:, :].rearrange("(n p) d -> n p d", p=P)

    NK = repeats * NT

    with ExitStack() as ctx:
        def bufs(name):
            return [
                ctx.enter_context(nc.sbuf_tensor(f"{name}{j}", [P, D], f32))
                for j in range(nb)
            ]

        sgt, mut, tgt, lt = bufs("sgt"), bufs("mut"), bufs("tgt"), bufs("lt")
        stats_t = ctx.enter_context(nc.sbuf_tensor("stats_t", [P, 2 * NT], f32))
        warm_t = ctx.enter_context(nc.sbuf_tensor("warm_t", [P, 1], f32))

        sg_sem = [ctx.enter_context(nc.semaphore(f"sg_sem{j}")) for j in range(nb)]
        mu_sem = [ctx.enter_context(nc.semaphore(f"mu_sem{j}")) for j in range(nb)]
        tg_sem = [ctx.enter_context(nc.semaphore(f"tg_sem{j}")) for j in range(nb)]
        asem = ctx.enter_context(nc.semaphore("asem"))
        vsem = ctx.enter_context(nc.semaphore("vsem"))
        ssem = ctx.enter_context(nc.semaphore("ssem"))
        block = ctx.enter_context(nc.Block())

        one_ap = nc.const_aps.tensor(1.0, (P, 1), f32)

        def on_act_ring(k):
            return k % 2 == 1   # odd iterations' target loads go via ACT ring

        @block.sync
        def _(sync):
            for k in range(NK):
                i, p = k % NT, k % nb
                if k >= nb:
                    # Square_{k-nb} read tgt[p] (w) and wrote mut[p] (q).
                    # prewarm shifts ticks by +1: Square_j completes at 3j+4.
                    sync.wait_ge(asem, 3 * (k - nb) + 4)
                sync.dma_start(out=mut[p][:, :], in_=mu3[i, :, :]).then_inc(mu_sem[p], 16)
                if not on_act_ring(k):
                    sync.dma_start(out=tgt[p][:, :], in_=tg3[i, :, :]).then_inc(tg_sem[p], 16)
            sync.wait_ge(asem, 3 * NK + 1)
            sync.dma_start(out=stats[:, :], in_=stats_t[:, :]).then_inc(ssem, 16)
            sync.wait_ge(ssem, 16)

        @block.scalar
        def _(scalar):
            # table prewarm overlaps the DMA fill            (tick 1)
            nc.scalar.activation(warm_t[:, :], one_ap, F.Ln).then_inc(asem, 1)
            for j in range(min(nb, NK)):
                nc.scalar.dma_start(
                    out=sgt[j][:, :], in_=sg3[j % NT, :, :]
                ).then_inc(sg_sem[j], 16)
                if on_act_ring(j):
                    nc.scalar.dma_start(
                        out=tgt[j][:, :], in_=tg3[j % NT, :, :]
                    ).then_inc(tg_sem[j], 16)
            for k in range(NK):
                i, p = k % NT, k % nb
                scalar.wait_ge(sg_sem[p], 16 * (k // nb + 1))
                nc.scalar.activation(
                    lt[p][:, :], sgt[p][:, :], F.Ln,
                    accum_out=stats_t[:, i : i + 1],
                ).then_inc(asem, 1)                         # tick 3k+2
                scalar.wait_ge(asem, 3 * k + 2)             # Ln_k done (RAW lt)
                nc.scalar.activation(
                    sgt[p][:, :], lt[p][:, :], F.Exp, scale=-0.5,
                ).then_inc(asem, 1)                         # tick 3k+3
                scalar.wait_ge(vsem, 2 * k + 2)             # mul_k done
                nc.scalar.activation(
                    mut[p][:, :], tgt[p][:, :], F.Square,
                    accum_out=stats_t[:, NT + i : NT + i + 1],
                ).then_inc(asem, 1)                         # tick 3k+4
                if k + nb < NK:
                    # sgt[p] free: sigma read by Ln_k, rs read by mul_k
                    # (completion-waited before Square_k).
                    nc.scalar.dma_start(
                        out=sgt[p][:, :], in_=sg3[(k + nb) % NT, :, :]
                    ).then_inc(sg_sem[p], 16)
                    if on_act_ring(k + nb):
                        # tgt[p]'s last reader is Square_k itself; the DMA may
                        # not start until it COMPLETES (issue order is not
                        # completion order).
                        scalar.wait_ge(asem, 3 * k + 4)
                        nc.scalar.dma_start(
                            out=tgt[p][:, :], in_=tg3[(k + nb) % NT, :, :]
                        ).then_inc(tg_sem[p], 16)

        @block.vector
        def _(vector):
            for k in range(NK):
                p = k % nb
                vector.wait_ge(mu_sem[p], 16 * (k // nb + 1))
                vector.wait_ge(tg_sem[p], 16 * (k // nb + 1))
                nc.vector.tensor_sub(
                    tgt[p][:, :], tgt[p][:, :], mut[p][:, :]
                ).then_inc(vsem, 1)                         # tick 2k+1
                vector.wait_ge(asem, 3 * k + 3)             # Exp_k done
                vector.wait_ge(vsem, 2 * k + 1)             # sub_k done (RAW)
                nc.vector.tensor_mul(
                    tgt[p][:, :], tgt[p][:, :], sgt[p][:, :]
                ).then_inc(vsem, 1)                         # tick 2k+2

    return nc


def build_nc_v3(repeats: int = 1, nb: int = 2, tw: int = 2) -> bass.Bass:
    """v3: like v2 but each DMA/compute chunk covers `tw` row-tiles
    (transfer size tw MB — halves per-transfer fixed costs and improves ring
    efficiency), plus an activation-table prewarm so the ~2.7us table load
    overlaps the DMA pipeline fill instead of serializing after it.
    """
    assert NT % tw == 0
    NC = NT // tw                # chunks per repeat
    FD = tw * D                  # free-dim elements per chunk
    nc = bass.Bass()
    f32 = mybir.dt.float32
    F = mybir.ActivationFunctionType
    mu = nc.dram_tensor("mu", [RPC, D], f32, kind="ExternalInput")
    sg = nc.dram_tensor("sigma", [RPC, D], f32, kind="ExternalInput")
    tg = nc.dram_tensor("target", [RPC, D], f32, kind="ExternalInput")
    stats = nc.dram_tensor("stats", [P, 2 * NC], f32, kind="ExternalOutput")

    # [p, n, d]: partition-major view; chunk c = [:, c*tw:(c+1)*tw, :]
    mu3 = mu[:, :].rearrange("(n p) d -> p n d", p=P)
    sg3 = sg[:, :].rearrange("(n p) d -> p n d", p=P)
    tg3 = tg[:, :].rearrange("(n p) d -> p n d", p=P)

    NK = repeats * NC

    with ExitStack() as ctx:
        def bufs(name):
            return [
                ctx.enter_context(nc.sbuf_tensor(f"{name}{j}", [P, tw, D], f32))
                for j in range(nb)
            ]

        sgt, mut, tgt, lt = bufs("sgt"), bufs("mut"), bufs("tgt"), bufs("lt")
        stats_t = ctx.enter_context(nc.sbuf_tensor("stats_t", [P, 2 * NC], f32))
        warm_t = ctx.enter_context(nc.sbuf_tensor("warm_t", [P, 1], f32))

        sg_sem = [ctx.enter_context(nc.semaphore(f"sg_sem{j}")) for j in range(nb)]
        mu_sem = [ctx.enter_context(nc.semaphore(f"mu_sem{j}")) for j in range(nb)]
        tg_sem = [ctx.enter_context(nc.semaphore(f"tg_sem{j}")) for j in range(nb)]
        asem = ctx.enter_context(nc.semaphore("asem"))
        vsem = ctx.enter_context(nc.semaphore("vsem"))
        ssem = ctx.enter_context(nc.semaphore("ssem"))
        block = ctx.enter_context(nc.Block())

        one_ap = nc.const_aps.tensor(1.0, (P, 1), f32)

        def chunk(t3, c):
            return t3[:, (c % NC) * tw : (c % NC) * tw + tw, :]

        @block.sync
        def _(sync):
            for k in range(NK):
                p = k % nb
                if k >= nb:
                    sync.wait_ge(asem, 3 * (k - nb) + 4)   # Square_{k-nb} done
                sync.dma_start(out=mut[p][:, :, :], in_=chunk(mu3, k)).then_inc(mu_sem[p], 16)
                sync.dma_start(out=tgt[p][:, :, :], in_=chunk(tg3, k)).then_inc(tg_sem[p], 16)
            sync.wait_ge(asem, 3 * NK + 1)
            sync.dma_start(out=stats[:, :], in_=stats_t[:, :]).then_inc(ssem, 16)
            sync.wait_ge(ssem, 16)

        @block.scalar
        def _(scalar):
            # table prewarm: no waits, so the ~2.7us ACT table load overlaps
            # the initial DMA fill. Reads an initialized const AP.
            nc.scalar.activation(
                warm_t[:, :], one_ap, F.Ln,
            ).then_inc(asem, 1)                             # tick 1
            for j in range(min(nb, NK)):
                nc.scalar.dma_start(
                    out=sgt[j][:, :, :], in_=chunk(sg3, j)
                ).then_inc(sg_sem[j], 16)
            for k in range(NK):
                i, p = k % NC, k % nb
                scalar.wait_ge(sg_sem[p], 16 * (k // nb + 1))
                nc.scalar.activation(
                    lt[p][:, :, :], sgt[p][:, :, :], F.Ln,
                    accum_out=stats_t[:, i : i + 1],
                ).then_inc(asem, 1)                         # tick 3k+2
                scalar.wait_ge(asem, 3 * k + 2)             # Ln_k done (RAW lt)
                nc.scalar.activation(
                    sgt[p][:, :, :], lt[p][:, :, :], F.Exp, scale=-0.5,
                ).then_inc(asem, 1)                         # tick 3k+3
                scalar.wait_ge(vsem, 2 * k + 2)             # mul_k done
                nc.scalar.activation(
                    mut[p][:, :, :], tgt[p][:, :, :], F.Square,
                    accum_out=stats_t[:, NC + i : NC + i + 1],
                ).then_inc(asem, 1)                         # tick 3k+4
                if k + nb < NK:
                    nc.scalar.dma_start(
                        out=sgt[p][:, :, :], in_=chunk(sg3, k + nb)
                    ).then_inc(sg_sem[p], 16)

        @block.vector
        def _(vector):
            for k in range(NK):
                p = k % nb
                vector.wait_ge(mu_sem[p], 16 * (k // nb + 1))
                vector.wait_ge(tg_sem[p], 16 * (k // nb + 1))
                nc.vector.tensor_sub(
                    tgt[p][:, :, :], tgt[p][:, :, :], mut[p][:, :, :]
                ).then_inc(vsem, 1)                         # tick 2k+1
                vector.wait_ge(asem, 3 * k + 3)             # Exp_k done
                vector.wait_ge(vsem, 2 * k + 1)             # sub_k done (RAW)
                nc.vector.tensor_mul(
                    tgt[p][:, :, :], tgt[p][:, :, :], sgt[p][:, :, :]
                ).then_inc(vsem, 1)                         # tick 2k+2

    return nc


def build_nc_v5(repeats: int = 1, nb: int = 3) -> bass.Bass:
    """v5: three balanced DMA streams, 16MB each per pass.
    SP ring: mu. ACT ring: sigma (issued from the ACT engine like v2).
    Pool SWDGE: target. Same compute pipeline as v2 (in-place DVE ops,
    activation accum_out row-sums, table prewarm)."""
    nc = bass.Bass()
    f32 = mybir.dt.float32
    F = mybir.ActivationFunctionType
    mu = nc.dram_tensor("mu", [RPC, D], f32, kind="ExternalInput")
    sg = nc.dram_tensor("sigma", [RPC, D], f32, kind="ExternalInput")
    tg = nc.dram_tensor("target", [RPC, D], f32, kind="ExternalInput")
    stats = nc.dram_tensor("stats", [P, 2 * NT], f32, kind="ExternalOutput")

    mu3 = mu[:, :].rearrange("(n p) d -> n p d", p=P)
    sg3 = sg[:, :].rearrange("(n p) d -> n p d", p=P)
    tg3 = tg[:, :].rearrange("(n p) d -> n p d", p=P)

    NK = repeats * NT

    with ExitStack() as ctx:
        def bufs(name):
            return [
                ctx.enter_context(nc.sbuf_tensor(f"{name}{j}", [P, D], f32))
                for j in range(nb)
            ]

        sgt, mut, tgt, lt = bufs("sgt"), bufs("mut"), bufs("tgt"), bufs("lt")
        stats_t = ctx.enter_context(nc.sbuf_tensor("stats_t", [P, 2 * NT], f32))
        warm_t = ctx.enter_context(nc.sbuf_tensor("warm_t", [P, 1], f32))

        sg_sem = [ctx.enter_context(nc.semaphore(f"sg_sem{j}")) for j in range(nb)]
        mu_sem = [ctx.enter_context(nc.semaphore(f"mu_sem{j}")) for j in range(nb)]
        tg_sem = [ctx.enter_context(nc.semaphore(f"tg_sem{j}")) for j in range(nb)]
        asem = ctx.enter_context(nc.semaphore("asem"))   # +1 per activation
        vsem = ctx.enter_context(nc.semaphore("vsem"))   # +1 per DVE op
        ssem = ctx.enter_context(nc.semaphore("ssem"))   # +16 final store
        block = ctx.enter_context(nc.Block())

        one_ap = nc.const_aps.tensor(1.0, (P, 1), f32)

        @block.sync
        def _(sync):
            for k in range(NK):
                i, p = k % NT, k % nb
                if k >= nb:
                    # Square_{k-nb} wrote mut[p] (q): slot free on completion.
                    # (prewarm shifts ticks by +1: Square_j completes at 3j+4)
                    sync.wait_ge(asem, 3 * (k - nb) + 4)
                sync.dma_start(out=mut[p][:, :], in_=mu3[i, :, :]).then_inc(mu_sem[p], 16)
            sync.wait_ge(asem, 3 * NK + 1)
            sync.dma_start(out=stats[:, :], in_=stats_t[:, :]).then_inc(ssem, 16)
            sync.wait_ge(ssem, 16)

        @block.gpsimd
        def _(gp):
            for k in range(NK):
                i, p = k % NT, k % nb
                if k >= nb:
                    # Square_{k-nb} read tgt[p] (w): last touch of the slot.
                    gp.wait_ge(asem, 3 * (k - nb) + 4)
                nc.gpsimd.dma_start(out=tgt[p][:, :], in_=tg3[i, :, :]).then_inc(tg_sem[p], 16)

        @block.scalar
        def _(scalar):
            # table prewarm overlaps the DMA fill              (tick 1)
            nc.scalar.activation(warm_t[:, :], one_ap, F.Ln).then_inc(asem, 1)
            for j in range(min(nb, NK)):
                nc.scalar.dma_start(
                    out=sgt[j][:, :], in_=sg3[j % NT, :, :]
                ).then_inc(sg_sem[j], 16)
            for k in range(NK):
                i, p = k % NT, k % nb
                scalar.wait_ge(sg_sem[p], 16 * (k // nb + 1))   # sigma_k landed
                nc.scalar.activation(
                    lt[p][:, :], sgt[p][:, :], F.Ln,
                    accum_out=stats_t[:, i : i + 1],
                ).then_inc(asem, 1)                             # tick 3k+2
                scalar.wait_ge(asem, 3 * k + 2)                 # Ln_k done (RAW lt)
                nc.scalar.activation(
                    sgt[p][:, :], lt[p][:, :], F.Exp, scale=-0.5,
                ).then_inc(asem, 1)                             # tick 3k+3
                scalar.wait_ge(vsem, 2 * k + 2)                 # mul_k done (w ready, rs dead)
                nc.scalar.activation(
                    mut[p][:, :], tgt[p][:, :], F.Square,
                    accum_out=stats_t[:, NT + i : NT + i + 1],
                ).then_inc(asem, 1)                             # tick 3k+4
                if k + nb < NK:
                    nc.scalar.dma_start(
                        out=sgt[p][:, :], in_=sg3[(k + nb) % NT, :, :]
                    ).then_inc(sg_sem[p], 16)

        @block.vector
        def _(vector):
            for k in range(NK):
                p = k % nb
                vector.wait_ge(mu_sem[p], 16 * (k // nb + 1))
                vector.wait_ge(tg_sem[p], 16 * (k // nb + 1))
                nc.vector.tensor_sub(
                    tgt[p][:, :], tgt[p][:, :], mut[p][:, :]
                ).then_inc(vsem, 1)                             # tick 2k+1
                vector.wait_ge(asem, 3 * k + 3)                 # Exp_k done (rs ready)
                vector.wait_ge(vsem, 2 * k + 1)                 # sub_k done (RAW)
                nc.vector.tensor_mul(
                    tgt[p][:, :], tgt[p][:, :], sgt[p][:, :]
                ).then_inc(vsem, 1)                             # tick 2k+2

    return nc


def build_nc_v2b(repeats: int = 1, nb: int = 3) -> bass.Bass:
    """v2 with BARRIERED repeats: each round re-runs the full single-shot
    body (per-round sigma prologue, per-round stats store) and round r+1
    starts only after round r's stats store completes (ssem). The
    differential slope over repeats then approximates the harness's
    single-shot exec time (fill + steady + tail), unlike the unbarriered
    builders whose repeats pipeline into pure steady-state."""
    nc = bass.Bass()
    f32 = mybir.dt.float32
    F = mybir.ActivationFunctionType
    mu = nc.dram_tensor("mu", [RPC, D], f32, kind="ExternalInput")
    sg = nc.dram_tensor("sigma", [RPC, D], f32, kind="ExternalInput")
    tg = nc.dram_tensor("target", [RPC, D], f32, kind="ExternalInput")
    stats = nc.dram_tensor("stats", [P, 2 * NT], f32, kind="ExternalOutput")

    mu3 = mu[:, :].rearrange("(n p) d -> n p d", p=P)
    sg3 = sg[:, :].rearrange("(n p) d -> n p d", p=P)
    tg3 = tg[:, :].rearrange("(n p) d -> n p d", p=P)

    NK = repeats * NT

    with ExitStack() as ctx:
        def bufs(name):
            return [
                ctx.enter_context(nc.sbuf_tensor(f"{name}{j}", [P, D], f32))
                for j in range(nb)
            ]

        sgt, mut, tgt, lt = bufs("sgt"), bufs("mut"), bufs("tgt"), bufs("lt")
        stats_t = ctx.enter_context(nc.sbuf_tensor("stats_t", [P, 2 * NT], f32))
        warm_t = ctx.enter_context(nc.sbuf_tensor("warm_t", [P, 1], f32))

        sg_sem = [ctx.enter_context(nc.semaphore(f"sg_sem{j}")) for j in range(nb)]
        mu_sem = [ctx.enter_context(nc.semaphore(f"mu_sem{j}")) for j in range(nb)]
        tg_sem = [ctx.enter_context(nc.semaphore(f"tg_sem{j}")) for j in range(nb)]
        asem = ctx.enter_context(nc.semaphore("asem"))
        vsem = ctx.enter_context(nc.semaphore("vsem"))
        ssem = ctx.enter_context(nc.semaphore("ssem"))
        block = ctx.enter_context(nc.Block())

        one_ap = nc.const_aps.tensor(1.0, (P, 1), f32)

        @block.sync
        def _(sync):
            for k in range(NK):
                r, t, p = k // NT, k % NT, k % nb
                if t == 0 and r > 0:
                    sync.wait_ge(ssem, 16 * r)
                if k >= nb:
                    sync.wait_ge(asem, 3 * (k - nb) + 4)
                sync.dma_start(out=mut[p][:, :], in_=mu3[t, :, :]).then_inc(mu_sem[p], 16)
                sync.dma_start(out=tgt[p][:, :], in_=tg3[t, :, :]).then_inc(tg_sem[p], 16)
                if t == NT - 1:
                    sync.wait_ge(asem, 3 * (k + 1) + 1)   # all ACT of round r done
                    sync.dma_start(out=stats[:, :], in_=stats_t[:, :]).then_inc(ssem, 16)
            sync.wait_ge(ssem, 16 * repeats)

        @block.scalar
        def _(scalar):
            nc.scalar.activation(warm_t[:, :], one_ap, F.Ln).then_inc(asem, 1)
            for k in range(NK):
                r, t, p = k // NT, k % NT, k % nb
                if t == 0:
                    if r > 0:
                        scalar.wait_ge(ssem, 16 * r)
                    for j in range(min(nb, NT)):
                        kk = k + j
                        nc.scalar.dma_start(
                            out=sgt[kk % nb][:, :], in_=sg3[j, :, :]
                        ).then_inc(sg_sem[kk % nb], 16)
                scalar.wait_ge(sg_sem[p], 16 * (k // nb + 1))
                nc.scalar.activation(
                    lt[p][:, :], sgt[p][:, :], F.Ln,
                    accum_out=stats_t[:, t : t + 1],
                ).then_inc(asem, 1)                             # tick 3k+2
                scalar.wait_ge(asem, 3 * k + 2)
                nc.scalar.activation(
                    sgt[p][:, :], lt[p][:, :], F.Exp, scale=-0.5,
                ).then_inc(asem, 1)                             # tick 3k+3
                scalar.wait_ge(vsem, 2 * k + 2)
                nc.scalar.activation(
                    mut[p][:, :], tgt[p][:, :], F.Square,
                    accum_out=stats_t[:, NT + t : NT + t + 1],
                ).then_inc(asem, 1)                             # tick 3k+4
                if t + nb < NT:
                    kk = k + nb
                    nc.scalar.dma_start(
                        out=sgt[kk % nb][:, :], in_=sg3[t + nb, :, :]
                    ).then_inc(sg_sem[kk % nb], 16)

        @block.vector
        def _(vector):
            for k in range(NK):
                r, p = k // NT, k % nb
                if k % NT == 0 and r > 0:
                    vector.wait_ge(ssem, 16 * r)
                vector.wait_ge(mu_sem[p], 16 * (k // nb + 1))
                vector.wait_ge(tg_sem[p], 16 * (k // nb + 1))
                nc.vector.tensor_sub(
                    tgt[p][:, :], tgt[p][:, :], mut[p][:, :]
                ).then_inc(vsem, 1)                             # tick 2k+1
                vector.wait_ge(asem, 3 * k + 3)
                vector.wait_ge(vsem, 2 * k + 1)
                nc.vector.tensor_mul(
                    tgt[p][:, :], tgt[p][:, :], sgt[p][:, :]
                ).then_inc(vsem, 1)                             # tick 2k+2

    return nc


def build_nc_v6(
    repeats: int = 1, nb: int = 3, nbs: int = 4, barrier: bool = False
) -> bass.Bass:
    """v6: tail-optimized pipeline.

    - DVE computes the quad row-sum with tensor_tensor_reduce
      (accum_out = sum(w*w)), so ACT runs only Ln+Exp and its stream
      never waits on the mu/target chain.
    - sigma moves on the Pool SWDGE ring (gated reloads happen on the
      otherwise-idle Pool engine), SP carries mu+target.
    - sigma gets nbs slots (deeper prefetch) so the sigma chain can run
      ahead; the post-DMA tail is just sub->mul->ttr on DVE.

    Ticks: asem: prewarm=1, Ln_k=2k+2, Exp_k=2k+3.
           vsem: sub_k=3k+1, mul_k=3k+2, ttr_k=3k+3.
    barrier=True serializes repeat rounds on the stats store (ssem) with
    per-round sigma prologues, mimicking single-shot timing per round.
    """
    nc = bass.Bass()
    f32 = mybir.dt.float32
    F = mybir.ActivationFunctionType
    A = mybir.AluOpType
    mu = nc.dram_tensor("mu", [RPC, D], f32, kind="ExternalInput")
    sg = nc.dram_tensor("sigma", [RPC, D], f32, kind="ExternalInput")
    tg = nc.dram_tensor("target", [RPC, D], f32, kind="ExternalInput")
    stats = nc.dram_tensor("stats", [P, 2 * NT], f32, kind="ExternalOutput")

    mu3 = mu[:, :].rearrange("(n p) d -> n p d", p=P)
    sg3 = sg[:, :].rearrange("(n p) d -> n p d", p=P)
    tg3 = tg[:, :].rearrange("(n p) d -> n p d", p=P)

    NK = repeats * NT

    with ExitStack() as ctx:
        def bufs(name, n):
            return [
                ctx.enter_context(nc.sbuf_tensor(f"{name}{j}", [P, D], f32))
                for j in range(n)
            ]

        sgt = bufs("sgt", nbs)
        mut, tgt = bufs("mut", nb), bufs("tgt", nb)
        lt = bufs("lt", 2)
        stats_t = ctx.enter_context(nc.sbuf_tensor("stats_t", [P, 2 * NT], f32))
        warm_t = ctx.enter_context(nc.sbuf_tensor("warm_t", [P, 1], f32))

        sg_sem = [ctx.enter_context(nc.semaphore(f"sg_sem{j}")) for j in range(nbs)]
        mu_sem = [ctx.enter_context(nc.semaphore(f"mu_sem{j}")) for j in range(nb)]
        tg_sem = [ctx.enter_context(nc.semaphore(f"tg_sem{j}")) for j in range(nb)]
        asem = ctx.enter_context(nc.semaphore("asem"))
        vsem = ctx.enter_context(nc.semaphore("vsem"))
        ssem = ctx.enter_context(nc.semaphore("ssem"))
        block = ctx.enter_context(nc.Block())

        one_ap = nc.const_aps.tensor(1.0, (P, 1), f32)

        @block.sync
        def _(sync):
            for k in range(NK):
                r, t, p = k // NT, k % NT, k % nb
                if barrier and t == 0 and r > 0:
                    sync.wait_ge(ssem, 16 * r)
                if k >= nb:
                    # ttr_{k-nb} read tgt[p] (w) and wrote mut[p] (dump)
                    sync.wait_ge(vsem, 3 * (k - nb) + 3)
                sync.dma_start(out=mut[p][:, :], in_=mu3[t, :, :]).then_inc(mu_sem[p], 16)
                sync.dma_start(out=tgt[p][:, :], in_=tg3[t, :, :]).then_inc(tg_sem[p], 16)
                if barrier and t == NT - 1:
                    sync.wait_ge(asem, 2 * (k + 1) + 1)   # all Ln/Exp of round
                    sync.wait_ge(vsem, 3 * (k + 1))       # all ttr of round
                    sync.dma_start(out=stats[:, :], in_=stats_t[:, :]).then_inc(ssem, 16)
            if not barrier:
                sync.wait_ge(asem, 2 * NK + 1)
                sync.wait_ge(vsem, 3 * NK)
                sync.dma_start(out=stats[:, :], in_=stats_t[:, :]).then_inc(ssem, 16)
                sync.wait_ge(ssem, 16)
            else:
                sync.wait_ge(ssem, 16 * repeats)

        @block.gpsimd
        def _(gp):
            if not barrier:
                for j in range(min(nbs, NK)):
                    nc.gpsimd.dma_start(
                        out=sgt[j][:, :], in_=sg3[j % NT, :, :]
                    ).then_inc(sg_sem[j], 16)
                for k in range(NK - nbs):
                    # reload evicts iter k's slot; rs dead after mul_k
                    gp.wait_ge(vsem, 3 * k + 2)
                    nc.gpsimd.dma_start(
                        out=sgt[(k + nbs) % nbs][:, :], in_=sg3[(k + nbs) % NT, :, :]
                    ).then_inc(sg_sem[(k + nbs) % nbs], 16)
            else:
                for r in range(repeats):
                    base = r * NT
                    if r > 0:
                        gp.wait_ge(ssem, 16 * r)
                    for j in range(min(nbs, NT)):
                        kk = base + j
                        nc.gpsimd.dma_start(
                            out=sgt[kk % nbs][:, :], in_=sg3[j, :, :]
                        ).then_inc(sg_sem[kk % nbs], 16)
                    for t in range(NT - nbs):
                        k = base + t
                        gp.wait_ge(vsem, 3 * k + 2)
                        nc.gpsimd.dma_start(
                            out=sgt[(k + nbs) % nbs][:, :], in_=sg3[t + nbs, :, :]
                        ).then_inc(sg_sem[(k + nbs) % nbs], 16)

        @block.scalar
        def _(scalar):
            nc.scalar.activation(warm_t[:, :], one_ap, F.Ln).then_inc(asem, 1)
            for k in range(NK):
                r, t, ps = k // NT, k % NT, k % nbs
                if barrier and t == 0 and r > 0:
                    scalar.wait_ge(ssem, 16 * r)    # round r-1 stats store read
                scalar.wait_ge(sg_sem[ps], 16 * (k // nbs + 1))
                nc.scalar.activation(
                    lt[k % 2][:, :], sgt[ps][:, :], F.Ln,
                    accum_out=stats_t[:, t : t + 1],
                ).then_inc(asem, 1)                             # tick 2k+2
                scalar.wait_ge(asem, 2 * k + 2)                 # Ln_k done (RAW)
                nc.scalar.activation(
                    sgt[ps][:, :], lt[k % 2][:, :], F.Exp, scale=-0.5,
                ).then_inc(asem, 1)                             # tick 2k+3

        @block.vector
        def _(vector):
            for k in range(NK):
                r, t, p = k // NT, k % NT, k % nb
                if barrier and t == 0 and r > 0:
                    vector.wait_ge(ssem, 16 * r)
                vector.wait_ge(mu_sem[p], 16 * (k // nb + 1))
                vector.wait_ge(tg_sem[p], 16 * (k // nb + 1))
                nc.vector.tensor_sub(
                    tgt[p][:, :], tgt[p][:, :], mut[p][:, :]
                ).then_inc(vsem, 1)                             # tick 3k+1
                vector.wait_ge(asem, 2 * k + 3)                 # Exp_k done (rs)
                vector.wait_ge(vsem, 3 * k + 1)                 # sub_k done (RAW)
                nc.vector.tensor_mul(
                    tgt[p][:, :], tgt[p][:, :], sgt[k % nbs][:, :]
                ).then_inc(vsem, 1)                             # tick 3k+2
                vector.wait_ge(vsem, 3 * k + 2)                 # mul_k done (RAW)
                nc.vector.tensor_tensor_reduce(
                    out=mut[p][:, :],
                    in0=tgt[p][:, :],
                    in1=tgt[p][:, :],
                    scale=1.0,
                    scalar=0.0,
                    op0=A.mult,
                    op1=A.add,
                    accum_out=stats_t[:, NT + t : NT + t + 1],
                ).then_inc(vsem, 1)                             # tick 3k+3

    return nc


def build_nc_v6b(repeats: int = 1, nb: int = 3, nbs: int = 4) -> bass.Bass:
    return build_nc_v6(repeats, nb=nb, nbs=nbs, barrier=True)


def build_nc_v7(
    repeats: int = 1, nb: int = 4, nbs: int = 4, S: int = 4
) -> bass.Bass:
    """v7: v6 with a short tail.

    - Last tile's mu/target DMAs and DVE chain split into S column
      sub-chunks of D/S, so the post-DMA critical chain is one short
      sub->mul->ttr on [P, D/S] instead of the full tile.
    - stats store issued from the ACT HWDGE ring (idle at round end)
      instead of queueing behind SP's data stream.
    - Rounds are always barriered (round r+1 gated on round r's stats
      store), so the differential slope over repeats equals single-shot
      time; repeats=1 is the production single-shot program.

    Tick bookkeeping is emission-time (python counters), not closed-form.
    """
    assert D % S == 0
    Dc = D // S
    NTm = NT - 1
    # stats columns: [0, NT) logdet per tile; [NT, NT+NTm) quad tiles 0..14;
    # [NT+NTm, NT+NTm+S) quad sub-chunks of tile 15.
    SW = NT + NTm + S
    nc = bass.Bass()
    f32 = mybir.dt.float32
    F = mybir.ActivationFunctionType
    A = mybir.AluOpType
    mu = nc.dram_tensor("mu", [RPC, D], f32, kind="ExternalInput")
    sg = nc.dram_tensor("sigma", [RPC, D], f32, kind="ExternalInput")
    tg = nc.dram_tensor("target", [RPC, D], f32, kind="ExternalInput")
    stats = nc.dram_tensor("stats", [P, SW], f32, kind="ExternalOutput")

    mu3 = mu[:, :].rearrange("(n p) d -> n p d", p=P)
    sg3 = sg[:, :].rearrange("(n p) d -> n p d", p=P)
    tg3 = tg[:, :].rearrange("(n p) d -> n p d", p=P)

    # ---- pre-pass: assign semaphore ticks in emission order ----
    # asem: prewarm -> 1; per round, per tile t: Ln, Exp.
    ln_done = {}
    exp_done = {}
    atick = 1
    for r in range(repeats):
        for t in range(NT):
            atick += 1
            ln_done[(r, t)] = atick
            atick += 1
            exp_done[(r, t)] = atick
    # vsem: per round: tiles 0..14: sub, mul, ttr; tile 15: per j: sub, mul, ttr.
    sub_done, mul_done, ttr_done = {}, {}, {}
    vtick = 0
    for r in range(repeats):
        for t in range(NTm):
            vtick += 1
            sub_done[(r, t)] = vtick
            vtick += 1
            mul_done[(r, t)] = vtick
            vtick += 1
            ttr_done[(r, t)] = vtick
        for j in range(S):
            vtick += 1
            sub_done[(r, NTm, j)] = vtick
            vtick += 1
            mul_done[(r, NTm, j)] = vtick
            vtick += 1
            ttr_done[(r, NTm, j)] = vtick
    vend = {r: ttr_done[(r, NTm, S - 1)] for r in range(repeats)}

    with ExitStack() as ctx:
        def bufs(name, n):
            return [
                ctx.enter_context(nc.sbuf_tensor(f"{name}{j}", [P, D], f32))
                for j in range(n)
            ]

        sgt = bufs("sgt", nbs)
        mut, tgt = bufs("mut", nb), bufs("tgt", nb)
        lt = bufs("lt", 2)
        stats_t = ctx.enter_context(nc.sbuf_tensor("stats_t", [P, SW], f32))
        warm_t = ctx.enter_context(nc.sbuf_tensor("warm_t", [P, 1], f32))

        sg_sem = [ctx.enter_context(nc.semaphore(f"sg_sem{j}")) for j in range(nbs)]
        mu_sem = [ctx.enter_context(nc.semaphore(f"mu_sem{j}")) for j in range(nb)]
        tg_sem = [ctx.enter_context(nc.semaphore(f"tg_sem{j}")) for j in range(nb)]
        m15 = [ctx.enter_context(nc.semaphore(f"m15_{j}")) for j in range(S)]
        t15 = [ctx.enter_context(nc.semaphore(f"t15_{j}")) for j in range(S)]
        asem = ctx.enter_context(nc.semaphore("asem"))
        vsem = ctx.enter_context(nc.semaphore("vsem"))
        ssem = ctx.enter_context(nc.semaphore("ssem"))
        block = ctx.enter_context(nc.Block())

        one_ap = nc.const_aps.tensor(1.0, (P, 1), f32)

        # slot of tile t in round r under continuous rotation (15 tiles/round
        # for mu/tg, NT tiles/round for sigma)
        def mslot(r, t):
            return (r * NTm + t) % nb

        def sslot(r, t):
            return (r * NT + t) % nbs

        # mu/tg load counts per slot, accumulated in emission order
        muld = [0] * nb

        @block.sync
        def _(sync):
            for r in range(repeats):
                if r > 0:
                    sync.wait_ge(ssem, 16 * r)
                for t in range(NTm):
                    p = mslot(r, t)
                    muld[p] += 1
                    # previous tenant: ttr of tile t-nb (same round) must be done
                    if t >= nb:
                        sync.wait_ge(vsem, ttr_done[(r, t - nb)])
                    sync.dma_start(out=mut[p][:, :], in_=mu3[t, :, :]).then_inc(mu_sem[p], 16)
                    sync.dma_start(out=tgt[p][:, :], in_=tg3[t, :, :]).then_inc(tg_sem[p], 16)
                # tile 15 sub-chunks into slot p15
                p15 = mslot(r, NTm)
                sync.wait_ge(vsem, ttr_done[(r, NTm - nb)])
                for j in range(S):
                    c = slice(j * Dc, (j + 1) * Dc)
                    sync.dma_start(out=mut[p15][:, c], in_=mu3[NTm, :, c]).then_inc(m15[j], 16)
                    sync.dma_start(out=tgt[p15][:, c], in_=tg3[NTm, :, c]).then_inc(t15[j], 16)
            sync.wait_ge(ssem, 16 * repeats)

        @block.gpsimd
        def _(gp):
            sgld = [0] * nbs
            for r in range(repeats):
                if r > 0:
                    gp.wait_ge(ssem, 16 * r)
                for j in range(min(nbs, NT)):
                    ps = sslot(r, j)
                    sgld[ps] += 1
                    nc.gpsimd.dma_start(
                        out=sgt[ps][:, :], in_=sg3[j, :, :]
                    ).then_inc(sg_sem[ps], 16)
                for t in range(NT - nbs):
                    # reload tile t+nbs evicts tile t's slot; rs dead after mul_t
                    # (tile 15's mul is per-subchunk; t+nbs<NT => t<=11 here)
                    gp.wait_ge(vsem, mul_done[(r, t)])
                    ps = sslot(r, t + nbs)
                    sgld[ps] += 1
                    nc.gpsimd.dma_start(
                        out=sgt[ps][:, :], in_=sg3[t + nbs, :, :]
                    ).then_inc(sg_sem[ps], 16)

        @block.scalar
        def _(scalar):
            nc.scalar.activation(warm_t[:, :], one_ap, F.Ln).then_inc(asem, 1)
            sgw = [0] * nbs
            for r in range(repeats):
                if r > 0:
                    scalar.wait_ge(ssem, 16 * r)   # round r-1 store read stats_t
                for t in range(NT):
                    ps = sslot(r, t)
                    sgw[ps] += 1
                    scalar.wait_ge(sg_sem[ps], 16 * sgw[ps])
                    nc.scalar.activation(
                        lt[t % 2][:, :], sgt[ps][:, :], F.Ln,
                        accum_out=stats_t[:, t : t + 1],
                    ).then_inc(asem, 1)
                    scalar.wait_ge(asem, ln_done[(r, t)])
                    nc.scalar.activation(
                        sgt[ps][:, :], lt[t % 2][:, :], F.Exp, scale=-0.5,
                    ).then_inc(asem, 1)
                # round-end stats store on the ACT HWDGE ring
                scalar.wait_ge(vsem, vend[r])       # all ttr accums of round r
                nc.scalar.dma_start(out=stats[:, :], in_=stats_t[:, :]).then_inc(ssem, 16)

        @block.vector
        def _(vector):
            tgw = [0] * nb
            for r in range(repeats):
                if r > 0:
                    vector.wait_ge(ssem, 16 * r)
                for t in range(NTm):
                    p = mslot(r, t)
                    tgw[p] += 1
                    ps = sslot(r, t)
                    vector.wait_ge(mu_sem[p], 16 * tgw[p])
                    vector.wait_ge(tg_sem[p], 16 * tgw[p])
                    nc.vector.tensor_sub(
                        tgt[p][:, :], tgt[p][:, :], mut[p][:, :]
                    ).then_inc(vsem, 1)
                    vector.wait_ge(asem, exp_done[(r, t)])
                    vector.wait_ge(vsem, sub_done[(r, t)])
                    nc.vector.tensor_mul(
                        tgt[p][:, :], tgt[p][:, :], sgt[ps][:, :]
                    ).then_inc(vsem, 1)
                    vector.wait_ge(vsem, mul_done[(r, t)])
                    nc.vector.tensor_tensor_reduce(
                        out=mut[p][:, :],
                        in0=tgt[p][:, :],
                        in1=tgt[p][:, :],
                        scale=1.0,
                        scalar=0.0,
                        op0=A.mult,
                        op1=A.add,
                        accum_out=stats_t[:, NT + t : NT + t + 1],
                    ).then_inc(vsem, 1)
                # tile 15 sub-chunks
                p15 = mslot(r, NTm)
                ps15 = sslot(r, NTm)
                for j in range(S):
                    c = slice(j * Dc, (j + 1) * Dc)
                    vector.wait_ge(m15[j], 16 * (r + 1))
                    vector.wait_ge(t15[j], 16 * (r + 1))
                    nc.vector.tensor_sub(
                        tgt[p15][:, c], tgt[p15][:, c], mut[p15][:, c]
                    ).then_inc(vsem, 1)
                    if j == 0:
                        vector.wait_ge(asem, exp_done[(r, NTm)])
                    vector.wait_ge(vsem, sub_done[(r, NTm, j)])
                    nc.vector.tensor_mul(
                        tgt[p15][:, c], tgt[p15][:, c], sgt[ps15][:, c]
                    ).then_inc(vsem, 1)
                    vector.wait_ge(vsem, mul_done[(r, NTm, j)])
                    col = NT + NTm + j
                    nc.vector.tensor_tensor_reduce(
                        out=mut[p15][:, c],
                        in0=tgt[p15][:, c],
                        in1=tgt[p15][:, c],
                        scale=1.0,
                        scalar=0.0,
                        op0=A.mult,
                        op1=A.add,
                        accum_out=stats_t[:, col : col + 1],
                    ).then_inc(vsem, 1)

    return nc


def build_nc_v9(
    repeats: int = 1, nb: int = 4, nbs: int = 4, S: int = 4, KT: int = 3
) -> bass.Bass:
    """v9: DMA-folded subtract + hybrid split tail.

    - mu_t lands in mbuf[p] (SP HWDGE); target_t is accum-DMA'd onto it
      with accum_op=subtract (Pool SWDGE): mbuf = mu - target, no engine
      op (the sign dies in the square). sigma also on the SP ring.
    - ACT: Ln (logdet accum) + Exp (rs = sigma^-1/2) per tile.
    - Full tiles: DVE mul (w = diff*rs, in place) + tensor_tensor_reduce
      (quad row-sum), 2 ops.
    - Trailing KT tiles are split into S column sub-chunks; since the
      accum dependency makes the stream end target-dense (arrivals faster
      than DVE's 2-pass rate), their square+row-sum runs on the ACT
      engine (Square with accum_out, idle after Exp15) while DVE does
      only the mul: the two engines pipeline the tail.
    - stats store on the ACT ring; rounds always barriered on ssem.
    """
    assert D % S == 0 and 1 <= KT < NT
    Dc = D // S
    NTf = NT - KT
    SW = NT + NTf + KT * S
    nc = bass.Bass()
    f32 = mybir.dt.float32
    F = mybir.ActivationFunctionType
    A = mybir.AluOpType
    mu = nc.dram_tensor("mu", [RPC, D], f32, kind="ExternalInput")
    sg = nc.dram_tensor("sigma", [RPC, D], f32, kind="ExternalInput")
    tg = nc.dram_tensor("target", [RPC, D], f32, kind="ExternalInput")
    stats = nc.dram_tensor("stats", [P, SW], f32, kind="ExternalOutput")

    mu3 = mu[:, :].rearrange("(n p) d -> n p d", p=P)
    sg3 = sg[:, :].rearrange("(n p) d -> n p d", p=P)
    tg3 = tg[:, :].rearrange("(n p) d -> n p d", p=P)

    units = [("full", t) for t in range(NTf)] + [
        ("sub", t, j) for t in range(NTf, NT) for j in range(S)
    ]

    def qcol(u):
        if u[0] == "full":
            return NT + u[1]
        return NT + NTf + (u[1] - NTf) * S + u[2]

    # ---- tick pre-pass ----
    # asem: prewarm=1; per round: Ln/Exp per tile, then KT*S Squares.
    ln_done, exp_done, sq_done = {}, {}, {}
    atick = 1
    for r in range(repeats):
        for t in range(NT):
            atick += 1
            ln_done[(r, t)] = atick
            atick += 1
            exp_done[(r, t)] = atick
        for u in units[NTf:]:
            atick += 1
            sq_done[(r, u)] = atick
    # vsem: full tiles mul+ttr; trailing units mul only.
    mul_done, ttr_done = {}, {}
    vtick = 0
    for r in range(repeats):
        for u in units:
            vtick += 1
            mul_done[(r, u)] = vtick
            if u[0] == "full":
                vtick += 1
                ttr_done[(r, u)] = vtick
    vend = {r: mul_done[(r, units[-1])] for r in range(repeats)}
    # last DVE touch of tile t's mbuf (full: ttr; sub tiles: last mul)
    last_v_of_tile = {}
    for r in range(repeats):
        for u in units:
            last_v_of_tile[(r, u[1])] = (
                ttr_done[(r, u)] if u[0] == "full" else mul_done[(r, u)]
            )

    with ExitStack() as ctx:
        def bufs(name, n):
            return [
                ctx.enter_context(nc.sbuf_tensor(f"{name}{j}", [P, D], f32))
                for j in range(n)
            ]

        sgt = bufs("sgt", nbs)
        mbuf = bufs("mbuf", nb)
        lt = bufs("lt", 2)
        dmp = ctx.enter_context(nc.sbuf_tensor("dmp", [P, D], f32))
        dmp2 = ctx.enter_context(nc.sbuf_tensor("dmp2", [P, KT * S * Dc], f32))
        stats_t = ctx.enter_context(nc.sbuf_tensor("stats_t", [P, SW], f32))
        warm_t = ctx.enter_context(nc.sbuf_tensor("warm_t", [P, 1], f32))

        sg_sem = [ctx.enter_context(nc.semaphore(f"sg_sem{j}")) for j in range(nbs)]
        mu_sem = [ctx.enter_context(nc.semaphore(f"mu_sem{j}")) for j in range(nb)]
        tg_sem = [ctx.enter_context(nc.semaphore(f"tg_sem{j}")) for j in range(nb)]
        # per-sub-chunk completion sems for split tiles (HWDGE completions
        # across dma_starts are unordered; counted shared sems can't tell
        # which sub-chunk landed)
        msub = [[ctx.enter_context(nc.semaphore(f"ms{k}_{j}")) for j in range(S)]
                for k in range(KT)]
        tsub = [[ctx.enter_context(nc.semaphore(f"ts{k}_{j}")) for j in range(S)]
                for k in range(KT)]
        asem = ctx.enter_context(nc.semaphore("asem"))
        vsem = ctx.enter_context(nc.semaphore("vsem"))
        ssem = ctx.enter_context(nc.semaphore("ssem"))
        block = ctx.enter_context(nc.Block())

        one_ap = nc.const_aps.tensor(1.0, (P, 1), f32)

        def mslot(r, t):
            return (r * NT + t) % nb

        def sslot(r, t):
            return (r * NT + t) % nbs

        def cols(u):
            if u[0] == "full":
                return slice(0, D)
            return slice(u[2] * Dc, (u[2] + 1) * Dc)

        @block.sync
        def _(sync):
            for r in range(repeats):
                if r > 0:
                    sync.wait_ge(ssem, 16 * r)
                for t in range(NT):
                    ps = sslot(r, t)
                    if t >= nbs:
                        prev = t - nbs
                        key = ("full", prev) if prev < NTf else ("sub", prev, S - 1)
                        sync.wait_ge(vsem, mul_done[(r, key)])
                    sync.dma_start(out=sgt[ps][:, :], in_=sg3[t, :, :]).then_inc(sg_sem[ps], 16)
                    p = mslot(r, t)
                    if t >= nb:
                        pt = t - nb
                        if pt < NTf:
                            sync.wait_ge(vsem, last_v_of_tile[(r, pt)])
                        else:
                            sync.wait_ge(asem, sq_done[(r, ("sub", pt, S - 1))])
                    if t < NTf:
                        sync.dma_start(out=mbuf[p][:, :], in_=mu3[t, :, :]).then_inc(mu_sem[p], 16)
                    else:
                        for j in range(S):
                            c = slice(j * Dc, (j + 1) * Dc)
                            sync.dma_start(out=mbuf[p][:, c], in_=mu3[t, :, c]).then_inc(msub[t - NTf][j], 16)
            sync.wait_ge(ssem, 16 * repeats)

        @block.gpsimd
        def _(gp):
            muw = [0] * nb
            for r in range(repeats):
                if r > 0:
                    gp.wait_ge(ssem, 16 * r)
                for u in units:
                    t = u[1]
                    p = mslot(r, t)
                    c = cols(u)
                    if u[0] == "full":
                        muw[p] += 1
                        gp.wait_ge(mu_sem[p], 16 * muw[p])
                        nc.gpsimd.dma_start(
                            out=mbuf[p][:, c], in_=tg3[t, :, c], accum_op=A.subtract
                        ).then_inc(tg_sem[p], 16)
                    else:
                        k, j = t - NTf, u[2]
                        gp.wait_ge(msub[k][j], 16 * (r + 1))
                        nc.gpsimd.dma_start(
                            out=mbuf[p][:, c], in_=tg3[t, :, c], accum_op=A.subtract
                        ).then_inc(tsub[k][j], 16)

        @block.scalar
        def _(scalar):
            nc.scalar.activation(warm_t[:, :], one_ap, F.Ln).then_inc(asem, 1)
            sgw = [0] * nbs
            for r in range(repeats):
                if r > 0:
                    scalar.wait_ge(ssem, 16 * r)
                for t in range(NT):
                    ps = sslot(r, t)
                    sgw[ps] += 1
                    scalar.wait_ge(sg_sem[ps], 16 * sgw[ps])
                    nc.scalar.activation(
                        lt[t % 2][:, :], sgt[ps][:, :], F.Ln,
                        accum_out=stats_t[:, t : t + 1],
                    ).then_inc(asem, 1)
                    scalar.wait_ge(asem, ln_done[(r, t)])
                    nc.scalar.activation(
                        sgt[ps][:, :], lt[t % 2][:, :], F.Exp, scale=-0.5,
                    ).then_inc(asem, 1)
                # trailing sub-chunks: square + row-sum on ACT
                for u in units[NTf:]:
                    t, j = u[1], u[2]
                    p = mslot(r, t)
                    c = cols(u)
                    col = qcol(u)
                    di = (t - NTf) * S + j
                    scalar.wait_ge(vsem, mul_done[(r, u)])
                    nc.scalar.activation(
                        dmp2[:, di * Dc : (di + 1) * Dc], mbuf[p][:, c], F.Square,
                        accum_out=stats_t[:, col : col + 1],
                    ).then_inc(asem, 1)
                scalar.wait_ge(asem, sq_done[(r, units[-1])])   # last Square accum landed
                nc.scalar.dma_start(out=stats[:, :], in_=stats_t[:, :]).then_inc(ssem, 16)

        @block.vector
        def _(vector):
            tgw = [0] * nb
            for r in range(repeats):
                if r > 0:
                    vector.wait_ge(ssem, 16 * r)
                for u in units:
                    t = u[1]
                    p = mslot(r, t)
                    ps = sslot(r, t)
                    c = cols(u)
                    if u[0] == "full":
                        tgw[p] += 1
                        vector.wait_ge(tg_sem[p], 16 * tgw[p])
                        vector.wait_ge(asem, exp_done[(r, t)])
                    else:
                        vector.wait_ge(tsub[t - NTf][u[2]], 16 * (r + 1))
                        if u[2] == 0:
                            vector.wait_ge(asem, exp_done[(r, t)])
                    nc.vector.tensor_mul(
                        mbuf[p][:, c], mbuf[p][:, c], sgt[ps][:, c]
                    ).then_inc(vsem, 1)
                    if u[0] == "full":
                        vector.wait_ge(vsem, mul_done[(r, u)])
                        nc.vector.tensor_tensor_reduce(
                            out=dmp[:, c],
                            in0=mbuf[p][:, c],
                            in1=mbuf[p][:, c],
                            scale=1.0,
                            scalar=0.0,
                            op0=A.mult,
                            op1=A.add,
                            accum_out=stats_t[:, NT + t : NT + t + 1],
                        ).then_inc(vsem, 1)

    return nc


def build_nc_v10(
    repeats: int = 1, nb: int = 4, nbs: int = 4, S: int = 4, KT: int = 3,
    lag: int = 2,
) -> bass.Bass:
    """v10: v2's proven op set, restructured for a clean tail.

    - Loads: SP HWDGE carries sigma+mu; Pool SWDGE carries target. The
      ACT engine issues no DMA at all (v2 issued sigma loads from ACT,
      stalling its compute stream on mu/target progress).
    - ACT: Ln_t (logdet accum), Exp_t (rs = sigma^-1/2), and Square_k
      (quad accum) emitted with a `lag`-tile delay so the sigma chain
      never waits on the mu/target chain.
    - DVE: sub_t (tgt -= mut, in place), mul_t (tgt *= rs, in place).
    - Trailing KT tiles split into S column sub-chunks (separate DMAs,
      per-sub sems, short DVE/ACT ops) => short post-stream tail.
    - stats store on the ACT ring after the last Square (same-engine
      completion wait); rounds barriered on ssem.
    """
    assert D % S == 0 and 1 <= KT < NT
    Dc = D // S
    NTf = NT - KT
    SW = NT + NTf + KT * S
    nc = bass.Bass()
    f32 = mybir.dt.float32
    F = mybir.ActivationFunctionType
    mu = nc.dram_tensor("mu", [RPC, D], f32, kind="ExternalInput")
    sg = nc.dram_tensor("sigma", [RPC, D], f32, kind="ExternalInput")
    tg = nc.dram_tensor("target", [RPC, D], f32, kind="ExternalInput")
    stats = nc.dram_tensor("stats", [P, SW], f32, kind="ExternalOutput")

    mu3 = mu[:, :].rearrange("(n p) d -> n p d", p=P)
    sg3 = sg[:, :].rearrange("(n p) d -> n p d", p=P)
    tg3 = tg[:, :].rearrange("(n p) d -> n p d", p=P)

    units = [("full", t) for t in range(NTf)] + [
        ("sub", t, j) for t in range(NTf, NT) for j in range(S)
    ]

    def qcol(u):
        if u[0] == "full":
            return NT + u[1]
        return NT + NTf + (u[1] - NTf) * S + u[2]

    # ---- tick pre-pass ----
    # ACT stream per round: Ln_t/Exp_t for t in 0..NT-1 with Square units
    # interleaved at a `lag`-tile delay, then remaining Squares, then store.
    # Build the ACT emission order first.
    act_stream = {}   # r -> list of ("ln",t) | ("exp",t) | ("sq",u)
    for r in range(repeats):
        st = []
        qi = 0            # next unit to square
        for t in range(NT):
            st.append(("ln", t))
            st.append(("exp", t))
            # append squares for units whose tile is <= t - lag
            while qi < len(units) and units[qi][1] <= t - lag:
                st.append(("sq", units[qi]))
                qi += 1
        while qi < len(units):
            st.append(("sq", units[qi]))
            qi += 1
        act_stream[r] = st

    ln_done, exp_done, sq_done = {}, {}, {}
    atick = 1
    for r in range(repeats):
        for ev in act_stream[r]:
            atick += 1
            if ev[0] == "ln":
                ln_done[(r, ev[1])] = atick
            elif ev[0] == "exp":
                exp_done[(r, ev[1])] = atick
            else:
                sq_done[(r, ev[1])] = atick
    # vsem: sub+mul per unit
    sub_done, mul_done = {}, {}
    vtick = 0
    for r in range(repeats):
        for u in units:
            vtick += 1
            sub_done[(r, u)] = vtick
            vtick += 1
            mul_done[(r, u)] = vtick
    # last Square touching tile t (Square writes mut dump / reads tgt)
    last_sq_of_tile = {}
    for r in range(repeats):
        for u in units:
            last_sq_of_tile[(r, u[1])] = sq_done[(r, u)]

    with ExitStack() as ctx:
        def bufs(name, n):
            return [
                ctx.enter_context(nc.sbuf_tensor(f"{name}{j}", [P, D], f32))
                for j in range(n)
            ]

        sgt = bufs("sgt", nbs)
        mut, tgt = bufs("mut", nb), bufs("tgt", nb)
        lt = bufs("lt", 2)
        stats_t = ctx.enter_context(nc.sbuf_tensor("stats_t", [P, SW], f32))
        warm_t = ctx.enter_context(nc.sbuf_tensor("warm_t", [P, 1], f32))

        sg_sem = [ctx.enter_context(nc.semaphore(f"sg_sem{j}")) for j in range(nbs)]
        mu_sem = [ctx.enter_context(nc.semaphore(f"mu_sem{j}")) for j in range(nb)]
        tg_sem = [ctx.enter_context(nc.semaphore(f"tg_sem{j}")) for j in range(nb)]
        msub = [[ctx.enter_context(nc.semaphore(f"ms{k}_{j}")) for j in range(S)]
                for k in range(KT)]
        tsub = [[ctx.enter_context(nc.semaphore(f"ts{k}_{j}")) for j in range(S)]
                for k in range(KT)]
        asem = ctx.enter_context(nc.semaphore("asem"))
        vsem = ctx.enter_context(nc.semaphore("vsem"))
        ssem = ctx.enter_context(nc.semaphore("ssem"))
        block = ctx.enter_context(nc.Block())

        one_ap = nc.const_aps.tensor(1.0, (P, 1), f32)

        def mslot(r, t):
            return (r * NT + t) % nb

        def sslot(r, t):
            return (r * NT + t) % nbs

        def cols(u):
            if u[0] == "full":
                return slice(0, D)
            return slice(u[2] * Dc, (u[2] + 1) * Dc)

        @block.sync
        def _(sync):
            for r in range(repeats):
                if r > 0:
                    sync.wait_ge(ssem, 16 * r)
                for t in range(NT):
                    ps = sslot(r, t)
                    if t >= nbs:
                        # rs in sgt dead after last mul of tile t-nbs
                        prev = t - nbs
                        key = ("full", prev) if prev < NTf else ("sub", prev, S - 1)
                        sync.wait_ge(vsem, mul_done[(r, key)])
                    sync.dma_start(out=sgt[ps][:, :], in_=sg3[t, :, :]).then_inc(sg_sem[ps], 16)
                    p = mslot(r, t)
                    if t >= nb:
                        # Square of tile t-nb wrote mut (dump) and read tgt (w)
                        sync.wait_ge(asem, last_sq_of_tile[(r, t - nb)])
                    if t < NTf:
                        sync.dma_start(out=mut[p][:, :], in_=mu3[t, :, :]).then_inc(mu_sem[p], 16)
                    else:
                        for j in range(S):
                            c = slice(j * Dc, (j + 1) * Dc)
                            sync.dma_start(out=mut[p][:, c], in_=mu3[t, :, c]).then_inc(msub[t - NTf][j], 16)
            sync.wait_ge(ssem, 16 * repeats)

        @block.gpsimd
        def _(gp):
            for r in range(repeats):
                if r > 0:
                    gp.wait_ge(ssem, 16 * r)
                for t in range(NT):
                    p = mslot(r, t)
                    if t >= nb:
                        gp.wait_ge(asem, last_sq_of_tile[(r, t - nb)])
                    if t < NTf:
                        nc.gpsimd.dma_start(
                            out=tgt[p][:, :], in_=tg3[t, :, :]
                        ).then_inc(tg_sem[p], 16)
                    else:
                        for j in range(S):
                            c = slice(j * Dc, (j + 1) * Dc)
                            nc.gpsimd.dma_start(
                                out=tgt[p][:, c], in_=tg3[t, :, c]
                            ).then_inc(tsub[t - NTf][j], 16)

        @block.scalar
        def _(scalar):
            nc.scalar.activation(warm_t[:, :], one_ap, F.Ln).then_inc(asem, 1)
            sgw = [0] * nbs
            for r in range(repeats):
                if r > 0:
                    scalar.wait_ge(ssem, 16 * r)
                for ev in act_stream[r]:
                    if ev[0] == "ln":
                        t = ev[1]
                        ps = sslot(r, t)
                        sgw[ps] += 1
                        scalar.wait_ge(sg_sem[ps], 16 * sgw[ps])
                        nc.scalar.activation(
                            lt[t % 2][:, :], sgt[ps][:, :], F.Ln,
                            accum_out=stats_t[:, t : t + 1],
                        ).then_inc(asem, 1)
                    elif ev[0] == "exp":
                        t = ev[1]
                        ps = sslot(r, t)
                        scalar.wait_ge(asem, ln_done[(r, t)])
                        nc.scalar.activation(
                            sgt[ps][:, :], lt[t % 2][:, :], F.Exp, scale=-0.5,
                        ).then_inc(asem, 1)
                    else:
                        u = ev[1]
                        t = u[1]
                        p = mslot(r, t)
                        c = cols(u)
                        col = qcol(u)
                        scalar.wait_ge(vsem, mul_done[(r, u)])
                        nc.scalar.activation(
                            mut[p][:, c], tgt[p][:, c], F.Square,
                            accum_out=stats_t[:, col : col + 1],
                        ).then_inc(asem, 1)
                scalar.wait_ge(asem, sq_done[(r, units[-1])])
                nc.scalar.dma_start(out=stats[:, :], in_=stats_t[:, :]).then_inc(ssem, 16)

        @block.vector
        def _(vector):
            tgw = [0] * nb
            for r in range(repeats):
                if r > 0:
                    vector.wait_ge(ssem, 16 * r)
                for u in units:
                    t = u[1]
                    p = mslot(r, t)
                    ps = sslot(r, t)
                    c = cols(u)
                    if u[0] == "full":
                        tgw[p] += 1
                        vector.wait_ge(mu_sem[p], 16 * tgw[p])
                        vector.wait_ge(tg_sem[p], 16 * tgw[p])
                    else:
                        vector.wait_ge(msub[t - NTf][u[2]], 16 * (r + 1))
                        vector.wait_ge(tsub[t - NTf][u[2]], 16 * (r + 1))
                    nc.vector.tensor_sub(
                        tgt[p][:, c], tgt[p][:, c], mut[p][:, c]
                    ).then_inc(vsem, 1)
                    if u[0] == "full" or u[2] == 0:
                        vector.wait_ge(asem, exp_done[(r, t)])
                    vector.wait_ge(vsem, sub_done[(r, u)])
                    nc.vector.tensor_mul(
                        tgt[p][:, c], tgt[p][:, c], sgt[ps][:, c]
                    ).then_inc(vsem, 1)

    return nc


BUILDER = build_nc_v2


def _get_nc() -> bass.Bass:
    global _nc_cache
    if _nc_cache is None:
        _nc_cache = BUILDER()
    return _nc_cache


def kernel(mu: np.ndarray, sigma: np.ndarray, target: np.ndarray) -> np.ndarray:
    global LAST_RESULTS
    mu = np.ascontiguousarray(np.asarray(mu, dtype=np.float32))
    sigma = np.ascontiguousarray(np.asarray(sigma, dtype=np.float32))
    target = np.ascontiguousarray(np.asarray(target, dtype=np.float32))
    assert mu.shape == (B, D) and sigma.shape == (B, D) and target.shape == (B, D)

    in_maps = []
    for c in range(N_CORES):
        s = slice(c * RPC, (c + 1) * RPC)
        in_maps.append({"mu": mu[s], "sigma": sigma[s], "target": target[s]})

    nc = _get_nc()
    res = run_bass_kernel_spmd(nc, in_maps, list(range(N_CORES)), trace=TRACE)
    LAST_RESULTS = res

    total = 0.0
    for r in res.results:
        total += float(r["stats"].astype(np.float64).sum())
    loss = 0.5 * (total / B + D * LOG_2PI)
    return np.asarray(loss, dtype=np.float32)

